# revision 42
# baseline (speedup 1.0000x reference)
"""Trainium2 Bass kernel for nn_Dilation2D (morphological dilation, max-plus conv).

    out[b,y,x,c] = max_{i,j} ( x_pad[b, y+i-1, x+j-1, c] + w[i,j,c] )

Sharding: pure data parallel over batch B=8 -> one image per NeuronCore.

Per-core layout: partitions p = hb*32 + c  (hb = one of 4 row-blocks of 128
image rows, c = channel).  Free dim = (row, x).  In this layout a tap is
    acc = max(x_tile[row+i, x+j] + w_vec[p], acc)
with w a per-partition scalar, so the adds are tensor_scalar/ACT-bias ops
and the maxes are 16-bit tensor_tensor ops in the DVE 2x perf mode.

The (y,x,c) -> ((hb,c),(y,x)) relayout rides the otherwise-idle TensorE:
big per-hb loads land [x-pos -> partitions, (xc,hb,c) -> free] (128B-
contiguous DRAM reads), PE transpose-mode matmuls flip each [128,128] tile
into PSUM, and the ACT PSUM->SBUF copy doubles as the f32->fp16 cast.
Reverse on the way out (fp16 transposes, ACT copy casts back to f32).
Tap work is split across DVE (tensor_scalar adds + all maxes) and ACT
(bias-adds).
"""

import numpy as np

import concourse.bass as bass
import concourse.bacc as bacc
import concourse.tile as tile
import concourse.dve_ops as dve_ops
from concourse import mybir
from concourse.bass_utils import run_bass_kernel_spmd
from concourse.dve_spec import Spec, Src0, Src1, C0, maxx, lower
from concourse.dve_uop import (
    DveOpSpec, UopConfig, UopDpConfig, InpSel, OutSel, OutPath, AluOp,
    AluInp, DelayInp, Trigger,
)

# Problem constants (hardcoded per contract).
B, H, W, C = 8, 512, 512, 32
KH, KW = 4, 4
HBLK = 4               # row blocks on partitions
HB = H // HBLK         # 128 rows per block
XC = W // 128          # 4 x-chunks of 128 pixels
YT = 8                 # output rows per chunk
NCHUNK = HB // YT      # 16 chunks
RT = YT + KH - 1       # 11 input rows per chunk (with halo)
XOFF = 16              # x=0 lives at column 16 (32B alignment for the xbar)
XW = 544               # padded row width: [15]=x=-1, [16,528)=x, [528,530)=halo
NEG = -60000.0         # -inf stand-in that fits fp16

F32 = mybir.dt.float32
F16 = mybir.dt.float16
AX = mybir.AluOpType

# tap routing. GPSIMD supports no float tensor ops at the ISA level on this
# toolchain, so taps live on DVE + ACT. Odd-j windows are 4B-aligned, so
# they run as one fused custom-DVE op each ((x+w) max acc, hand-written
# 2x_1p uop program); one even-j tap uses DVE tensor_scalar (2x_2p) + max;
# the remaining even-j adds ride ACT's activation bias (alignment-blind).
DVE_TAPS = [(0, 1), (1, 1), (2, 1), (3, 1),      # odd j: fused custom op
            (0, 3), (1, 3), (2, 3), (3, 3)]
DVE_TS_TAPS = [(0, 0)]                           # even j: TS add + TT max
ACT_TAPS = [
    (i, j) for i in range(KH) for j in range(KW)
    if (i, j) not in DVE_TAPS and (i, j) not in DVE_TS_TAPS
]

_ENABLE, _DISABLE = 1, 0


def _dp_stage(op, a, b, cap_lanes=(), pass_lanes=()):
    delay = [DelayInp.PREV_ALU_OUT] * 7
    enable = [_DISABLE] * 7
    for ln in pass_lanes:
        delay[ln] = DelayInp.PREV_DELAY
        enable[ln] = _ENABLE
    for ln in cap_lanes:
        delay[ln] = DelayInp.PREV_ALU_OUT
        enable[ln] = _ENABLE
    return UopDpConfig(
        op=op, alu_src0=a, alu_src1=b,
        delay=delay, alu_out_enable=_ENABLE, swap_enable=_DISABLE,
        alu_out_a_enable=_DISABLE, alu_out_b_enable=_DISABLE,
        delay_enable=enable, idx0_sel=0, idx1_sel=0,
    )


def _build_2x_uop():
    """2x_1p program for out = max(src0 + c0, src1): lo pair on slices 0/2,
    hi pair on slices 1/3, packed write via WR0_LO/WR0_HI."""
    inp = [InpSel.ZERO] * 8
    inp_en = [_DISABLE] * 8
    for ln, sel in {0: InpSel.SRC_0, 1: InpSel.CONST_0, 2: InpSel.SRC_1,
                    3: InpSel.SRC_0_HI, 4: InpSel.SRC_1_HI}.items():
        inp[ln + 1] = sel
        inp_en[ln + 1] = _ENABLE
    D = AluInp
    dp = [
        _dp_stage(AluOp.ADD, D.PREV_DELAY_0, D.PREV_DELAY_1,
                  pass_lanes=(0, 1, 2, 3, 4)),
        _dp_stage(AluOp.ADD, D.PREV_DELAY_3, D.PREV_DELAY_1,
                  cap_lanes=(5,), pass_lanes=(1, 2, 4)),
        _dp_stage(AluOp.MAX, D.PREV_DELAY_5, D.PREV_DELAY_2,
                  cap_lanes=(0,), pass_lanes=(4,)),
        _dp_stage(AluOp.MAX, D.PREV_DELAY_0, D.PREV_DELAY_4,
                  cap_lanes=(1,)),
        _dp_stage(AluOp.BYPASS, D.PREV_ALU_OUT, D.PREV_ALU_OUT,
                  pass_lanes=(1,)),
        _dp_stage(AluOp.BYPASS, D.PREV_ALU_OUT, D.PREV_ALU_OUT,
                  pass_lanes=(1,)),
        _dp_stage(AluOp.BYPASS, D.PREV_ALU_OUT, D.PREV_ALU_OUT,
                  pass_lanes=(1,)),
        _dp_stage(AluOp.BYPASS, D.PREV_ALU_OUT, D.PREV_ALU_OUT,
                  pass_lanes=(1,)),
    ]
    out = {OutPath.WR0_LO: OutSel.DELAY_1, OutPath.WR0_HI: OutSel.ALU_OUT,
           OutPath.WR1_LO: OutSel.ALU_OUT, OutPath.WR1_HI: OutSel.ALU_OUT}
    out_en = {OutPath.WR0_LO: _ENABLE, OutPath.WR0_HI: _ENABLE,
              OutPath.WR1_LO: _DISABLE, OutPath.WR1_HI: _DISABLE}
    return UopConfig(
        datapath_config=dp, inp=inp, inp_enable=inp_en,
        out=out, out_enable=out_en, require_inp0=1, require_inp1=1,
        trigger=(Trigger.SRC_TENSOR_DONE, Trigger.NONE, Trigger.NONE),
        next_uop=(0, 0, 0), repeat_count=0,
    )


def _register_tap_op():
    """Register TAP_MAXADD_ANT (out = max(in0 + s0, in1)) with a base 1x
    program from lower() and the hand-written 2x_1p slot."""
    name = "TAP_MAXADD_ANT"
    for o in dve_ops.OPS:
        if o.name == name:
            return o
    def _ref(in0, in1, s0, s1, imm2):
        s0 = np.asarray(s0)
        if s0.ndim and s0.ndim < in0.ndim:
            s0 = s0.reshape(s0.shape[0], *([1] * (in0.ndim - 1)))
        in1 = np.asarray(in1).reshape(in0.shape)
        return np.maximum(in0.astype(np.float32) + s0, in1)

    spec = Spec(body=maxx(Src0 + C0, Src1), reference=_ref)
    row = dve_ops._CUSTOM_DVE_ROW_BASE + len(dve_ops.OPS)
    u2 = _build_2x_uop()
    u2.validate("v3")
    full = DveOpSpec(name=name, opcode=row, uops=lower(spec, ver="v3"),
                     uops_2x=[u2], rd1_en=True, perf_max=1)
    full.validate("v3")
    op = dve_ops.DveOp(name, spec, subdim=False,
                       uops_sha={"v3": full.sha("v3")})
    dve_ops.OPS.append(op)
    dve_ops._SUB_OPCODE_FOR_NAME[name] = row
    dve_ops.CUSTOM_DVE_SPECS[name] = spec
    dve_ops._COMPILE_CACHE[(name, "v3")] = full
    return op


def _build_program(repeat: int = 1):
    tap_op = _register_tap_op()
    nc = bacc.Bacc("TRN2", target_bir_lowering=False, debug=False)
    x_d = nc.dram_tensor("x", [H, W, C], F32, kind="ExternalInput").ap()
    w_d = nc.dram_tensor("w", [KH, KW, C], F32, kind="ExternalInput").ap()
    o_d = nc.dram_tensor("out", [H, W, C], F32, kind="ExternalOutput").ap()

    # DRAM views: q = x-position within an x-chunk; dims [q, y, xc, hb, c]
    x_v = x_d.rearrange("(hb y) (xc q) c -> q y xc hb c", hb=HBLK, q=128)
    o_v = o_d.rearrange("(hb y) (xc q) c -> q y xc hb c", hb=HBLK, q=128)

    with tile.TileContext(nc) as tc:
        consts = tc.alloc_tile_pool(name="consts", bufs=1)
        w_sb = consts.tile([128, KH * KW], F32)
        w_r = w_d.rearrange("i j c -> c (i j)")
        for hb in range(HBLK):
            nc.sync.dma_start(out=w_sb[32 * hb : 32 * (hb + 1), :], in_=w_r)
        import concourse.masks as masks
        id32 = consts.tile([128, 128], F32)
        masks.make_identity(nc, id32[:])
        id16 = consts.tile([128, 128], F16)
        masks.make_identity(nc, id16[:])

        pre32_pool = tc.alloc_tile_pool(name="pre32", bufs=2)
        xbuf_pool = tc.alloc_tile_pool(name="xbuf", bufs=2)
        tap_pool = tc.alloc_tile_pool(name="taps", bufs=4)
        acc_pool = tc.alloc_tile_pool(name="acc", bufs=2)
        ost_pool = tc.alloc_tile_pool(name="ost", bufs=2)
        psi_pool = tc.alloc_tile_pool(name="psi", bufs=3, space="PSUM")
        pso_pool = tc.alloc_tile_pool(name="pso", bufs=3, space="PSUM")

        for ck_rep in range(NCHUNK * repeat):
            ck = ck_rep % NCHUNK
            y0 = ck * YT  # first output row (within each hb block)

            # ---- load: DRAM -> pre32 [q, (r, xc, hb, c)], one big DMA per
            # hb ((y,xc) and (r,xc) merge, keeping APs at 3 dims) ----
            r_lo = 1 if ck == 0 else 0
            r_hi = RT - 2 if ck == NCHUNK - 1 else RT
            pre32 = pre32_pool.tile([128, RT * XC * HBLK * C], F32)
            p32v = pre32[:].rearrange(
                "q (r xc hb c) -> q r xc hb c", r=RT, xc=XC, hb=HBLK, c=C
            )
            for hb in range(HBLK):
                nc.sync.dma_start(
                    out=p32v[:, r_lo:r_hi, :, hb],
                    in_=x_v[:, y0 - 1 + r_lo : y0 - 1 + r_hi, :, hb],
                )
                if ck == 0:
                    # r=0 is y_loc=-1: row 127 of block hb-1 (hb=0 gets a
                    # dummy row; masked to NEG after the relayout)
                    nc.sync.dma_start(
                        out=p32v[:, 0, :, hb],
                        in_=x_v[:, HB - 1 if hb else 0, :, max(hb - 1, 0)],
                    )
                if ck == NCHUNK - 1:
                    # r in {RT-2, RT-1} are y_loc {128,129}: rows 0,1 of hb+1
                    # (hb=3 gets dummy rows; masked to NEG after)
                    nc.sync.dma_start(
                        out=p32v[:, RT - 2 : RT, :, hb],
                        in_=x_v[:, 0:2, :, min(hb + 1, HBLK - 1)],
                    )

            # ---- relayout: PE transpose-mode matmuls [q,(hb,c)]->[(hb,c),q]
            # into PSUM; ACT copies PSUM -> x-tile, casting f32 -> fp16 ----
            xt = xbuf_pool.tile([128, RT * XW], F16)
            xt_v = xt[:].rearrange("p (r x) -> p r x", r=RT, x=XW)
            for r in range(RT):
                ps = psi_pool.tile([128, XC * 128], F32)  # one PSUM bank
                for xc in range(XC):
                    nc.tensor.matmul(
                        ps[:, 128 * xc : 128 * (xc + 1)],
                        p32v[:, r, xc],
                        id32[:],
                        start=(xc == 0),
                        stop=(xc == XC - 1),
                        is_transpose=True,
                        skip_group_check=True,
                    )
                nc.scalar.copy(xt_v[:, r, XOFF : XOFF + W], ps[:])
            # borders: left halo col (x=-1), right cols [528, 544)
            nc.gpsimd.memset(xt_v[:, :, XOFF - 1 : XOFF], NEG)
            nc.gpsimd.memset(xt_v[:, :, XOFF + W :], NEG)
            if ck == 0:
                nc.gpsimd.memset(xt_v[0:32, 0, :], NEG)          # hb=0, y=-1
            if ck == NCHUNK - 1:
                nc.gpsimd.memset(xt_v[96:128, RT - 2 : RT, :], NEG)  # hb=3

            def win(i, j):
                return xt_v[:, i : i + YT, XOFF - 1 + j : XOFF - 1 + j + W]

            def w_ap(i, j):
                t = i * KW + j
                return w_sb[:, t : t + 1]

            # ---- taps ----
            acc = acc_pool.tile([128, YT * W], F16, tag="acc_dve")
            acc_v = acc[:].rearrange("p (r x) -> p r x", r=YT, x=W)

            # DVE chain: first tap straight into acc (tensor_scalar add),
            # remaining odd-j taps as one fused custom op each.
            (i0, j0) = DVE_TAPS[0]
            nc.vector.tensor_scalar(
                acc_v[:], win(i0, j0), w_ap(i0, j0), None, AX.add
            )
            for (i, j) in DVE_TAPS[1:]:
                inst = nc.vector._custom_dve(
                    tap_op, out=acc_v[:], in0=win(i, j), in1=acc_v[:],
                    s0=w_ap(i, j),
                )
                inst.perf_max = 1
            for (i, j) in DVE_TS_TAPS:
                t16 = tap_pool.tile([128, YT * W], F16)
                t16v = t16[:].rearrange("p (r x) -> p r x", r=YT, x=W)
                nc.vector.tensor_scalar(
                    t16v[:], win(i, j), w_ap(i, j), None, AX.add
                )
                nc.vector.tensor_tensor(
                    out=acc_v[:], in0=acc_v[:], in1=t16v[:], op=AX.max
                )
            for (i, j) in ACT_TAPS:
                t16 = tap_pool.tile([128, YT * W], F16)
                t16v = t16[:].rearrange("p (r x) -> p r x", r=YT, x=W)
                nc.scalar.add(t16v[:], win(i, j), w_ap(i, j))
                nc.vector.tensor_tensor(
                    out=acc_v[:], in0=acc_v[:], in1=t16v[:], op=AX.max
                )

            # ---- transpose back on PE (fp16), ACT copy casts to f32, store
            ost32 = ost_pool.tile([128, YT * XC * HBLK * C], F32, tag="o32")
            o32v = ost32[:].rearrange(
                "q (r xc hb c) -> q r xc hb c", r=YT, xc=XC, hb=HBLK, c=C
            )
            for r in range(YT):
                ps = pso_pool.tile([128, XC * 128], F16)  # half a PSUM bank
                for xc in range(XC):
                    nc.tensor.matmul(
                        ps[:, 128 * xc : 128 * (xc + 1)],
                        acc_v[:, r, 128 * xc : 128 * (xc + 1)],
                        id16[:],
                        start=(xc == 0),
                        stop=(xc == XC - 1),
                        is_transpose=True,
                        skip_group_check=True,
                    )
                nc.scalar.copy(o32v[:, r], ps[:])
            for hb in range(HBLK):
                nc.sync.dma_start(
                    out=o_v[:, y0 : y0 + YT, :, hb], in_=o32v[:, :, :, hb]
                )

        for p in (pso_pool, psi_pool, ost_pool, acc_pool, tap_pool,
                  xbuf_pool, pre32_pool, consts):
            p.release()

    nc.compile()
    return nc


_CACHED = {}


def _get_program(repeat: int = 1):
    if repeat not in _CACHED:
        _CACHED[repeat] = _build_program(repeat)
    return _CACHED[repeat]


def kernel(x: np.ndarray, w: np.ndarray, _trace: bool = False,
           _repeat: int = 1):
    """Full inputs in, full output out. Shards batch across 8 cores."""
    x = np.ascontiguousarray(np.asarray(x), dtype=np.float32)
    w = np.ascontiguousarray(np.asarray(w), dtype=np.float32)
    assert x.shape == (B, H, W, C) and w.shape == (KH, KW, C)
    nc = _get_program(_repeat)
    core_ids = list(range(B))
    in_maps = [{"x": x[b], "w": w} for b in range(B)]
    res = run_bass_kernel_spmd(nc, in_maps, core_ids, trace=_trace)
    out = np.stack([res.results[i]["out"] for i in range(B)], axis=0)
    if _trace:
        kernel.last_exec_time_ns = res.exec_time_ns
        kernel.last_results = res
    return out


if __name__ == "__main__":
    rng = np.random.default_rng(0)
    x = rng.standard_normal((B, H, W, C), dtype=np.float32)
    w = (rng.standard_normal((KH, KW, C)) * 0.1).astype(np.float32)
    out = kernel(x, w)
    print("out", out.shape, out.dtype, float(out.mean()))
